# revision 6
# baseline (speedup 1.0000x reference)
"""Trainium2 Bass kernel for the class-balanced supervised-contrastive loss.

Math (reference semantics, shift-invariant form with constant shift 10):
  l_ij = (f_i . g_j) / T,  T = 0.1, g = [features; centers; features_ood]
  E_ij = exp(l_ij - 10)
  S_i  = sum_{j != i} E_ij / (w_j - eq_ij)        (w_j = class count, eq = label match)
  P_i  = sum_{j != i} eq_ij (l_ij - 10)
  loss = -mean_i( P_i / K_i - log S_i ),  K_i = batch count of class t_i

Key identity: for matched columns j (eq_ij = 1) the class equals t_i, so w_j is a
per-row constant w*. The device therefore only needs, per row:
  A_i  = sum_j exp(10*(r_ij + bias1_j))      bias1_j = (ln(1/w_j) - 10)/10
  S2_i = sum_j eq_ij * E1_ij                 (E1 = the summand of A)
  S3_i = sum_j eq_ij * psum_ij               (psum = 1024*(r + bias1))
  diag = psum_ii                             (self column, for exclusion terms)
Everything else is O(B) host math.

Matmuls run in fp8 e4m3 DoubleRow mode (0.5 PE cycles/col, two K-slices per
instruction). Operands are scaled by 32 on the host, so psum = 1024*r and the
ACT exp uses scale 10/1024. Columns outside the matched window use a single
fh.gh term (random ~2e-2 error per exp term, averages out over ~9k summed
terms); the matched window gets fh.gh + fl.gh + fh.gl (logit error ~7e-4),
with the lo terms narrowed to the 256 columns where matches live. The
per-column bias enters as a K=2 bf16 matmul of ones against the hi/lo pair of
1024*bias1 (error ~1e-5). End-to-end numpy emulation: rel err 2.3e-5.

Device layout per core (rows sharded, 512 rows/core, 4 row tiles):
  per row tile: 5 psum groups of <=4 column chunks ([128,2048] = 4 banks,
  2-buffer ping-pong fills all 8 banks). Group 0 holds the window chunks.
  ACT: E1 = exp(psum*10/1024) with accum_out -> A partial per group
  DVE: scalar_tensor_tensor (ta == t_i) * {E1, psum} -> S2/S3; ident*psum diag
  Bulk g-matrix DMAs ride the otherwise-idle Pool (gpsimd) queue.
"""

import ml_dtypes
import numpy as np

import concourse.bass as bass
import concourse.mybir as mybir
import concourse.tile as tile
from concourse.bass_utils import run_bass_kernel_spmd

NCORES = 8
C, TEMP = 1000, 0.1
B, BO, D = 4096, 4096, 512
N = B + C + BO              # 9192
NPAD = 9216                 # 18 * 512
PAD = NPAD - N
NCH = NPAD // 512           # 18 column chunks
RPC = B // NCORES           # 512 rows per core
MT = RPC // 128             # 4 row tiles per core
SF = 32.0                   # fp8 operand scale; psum = SF*SF*(r + bias1)
PS = SF * SF                # 1024

F32 = mybir.dt.float32
BF16 = mybir.dt.bfloat16
FP8 = mybir.dt.float8e4
ALU = mybir.AluOpType
AF = mybir.ActivationFunctionType
BFNP = ml_dtypes.bfloat16
F8NP = ml_dtypes.float8_e4m3   # TRN FP8_EXP4: max +-240, then inf
DR = mybir.MatmulPerfMode.DoubleRow

# This walrus build accepts only one sync-wait command per engine instruction.
# Move surplus waits onto standalone EventSemaphore instructions just before
# the affected instruction (same engine, so blocking semantics are identical).
_SPLIT_SKIP = ("InstEventSemaphore",)


def _split_multi_waits(nc):
    n = 0
    for f in nc.m.functions:
        for bb in f.blocks:
            new = []
            for ins in bb.instructions:
                si = ins.sync_info
                if (
                    si is not None
                    and si.on_wait
                    and len(si.on_wait) > 1
                    and type(ins).__name__ not in _SPLIT_SKIP
                ):
                    waits = list(si.on_wait)
                    for w in waits[:-1]:
                        es = mybir.InstEventSemaphore(
                            name=f"wsplit_{n}",
                            engine=ins.engine,
                            sync_info=mybir.SyncInfo(on_wait=[w], on_update=[]),
                        )
                        n += 1
                        new.append(es)
                    ins.sync_info = mybir.SyncInfo(
                        on_wait=[waits[-1]], on_update=list(si.on_update)
                    )
                new.append(ins)
            bb.instructions = new
    return n


def _build_nc(eqw=2, corr_last=None, woff=None):
    assert eqw <= 4, "window chunks must fit in psum group 0"
    nc = bass.Bass()
    # host pre-tiles to the SBUF layout: col chunk ch lives at [128, 2048]
    # block ch with inner offset 512*k + j  (k = contraction slice)
    g8h = nc.declare_dram_parameter("g8h", [128, NCH * 2048], FP8, isOutput=False)
    g8l = nc.declare_dram_parameter("g8l", [128, eqw * 2048], FP8, isOutput=False)
    f8h = nc.declare_dram_parameter("f8h", [128, 2048], FP8, isOutput=False)
    f8l = nc.declare_dram_parameter("f8l", [128, 2048], FP8, isOutput=False)
    # [2, .]: row 0 = (ones128, hi of 1024*bias1), row 1 = (ones128, lo part)
    cst = nc.declare_dram_parameter("cst", [2, 128 + NPAD], BF16, isOutput=False)
    ta = nc.declare_dram_parameter("ta", [128, eqw * 512], F32, isOutput=False)
    tvec = nc.declare_dram_parameter("tvec", [128, MT], F32, isOutput=False)
    ident = nc.declare_dram_parameter("ident", [128, 128], F32, isOutput=False)
    out = nc.declare_dram_parameter("out", [128, 4 * MT], F32, isOutput=True)

    # column chunks packed into psum groups sized [4,4,4,3,3]: every ACT
    # drain is then >=1652ns, which covers the PE fill of the group two
    # slots ahead (2-buffer PSUM ping-pong never stalls the pipeline).
    # Emission order per row tile puts the window group (expensive fill,
    # most input tensors) last and the tail chunks (smallest DMA) first.
    groups = [[0, 1, 2, 3], [4, 5, 6, 7], [8, 9, 10, 11],
              [12, 13, 14], [15, 16, 17]]
    emit_order = [4, 3, 1, 2, 0]
    NG = len(groups)

    with tile.TileContext(nc) as tc:
        with (
            tc.tile_pool(name="const", bufs=1) as const,
            tc.tile_pool(name="stats", bufs=1) as stats,
            tc.tile_pool(name="e1", bufs=3) as e1p,
            tc.tile_pool(name="scr", bufs=2) as scr,
            tc.tile_pool(name="psum", bufs=2, space="PSUM") as psp,
        ):
            # preload the exp spline table while DMAs stream in
            warm = const.tile([128, 1], F32)
            nc.vector.memset(warm[:], 0.0)
            nc.scalar.activation(warm[:], warm[:], AF.Exp)

            f8h_sb = const.tile([128, 4, 512], FP8)
            f8l_sb = const.tile([128, 4, 512], FP8)
            cst_sb = const.tile([2, 128 + NPAD], BF16)
            # SP queue, in consumption order: f rows, then bias columns for
            # the first-emitted groups (tail chunks), then the rest.
            nc.sync.dma_start(out=f8h_sb[:], in_=f8h[:])
            nc.sync.dma_start(out=cst_sb[:, 0:128], in_=cst[:, 0:128])
            nc.sync.dma_start(
                out=cst_sb[:, 128 + 6144 :], in_=cst[:, 128 + 6144 :]
            )
            nc.sync.dma_start(
                out=cst_sb[:, 128 + 2048 : 128 + 6144],
                in_=cst[:, 128 + 2048 : 128 + 6144],
            )
            nc.sync.dma_start(
                out=cst_sb[:, 128 : 128 + 2048], in_=cst[:, 128 : 128 + 2048]
            )
            nc.sync.dma_start(out=f8l_sb[:], in_=f8l[:])
            ta_sb = const.tile([128, eqw * 512], F32)
            tvec_sb = const.tile([128, MT], F32)
            ident_sb = const.tile([128, 128], F32)
            nc.sync.dma_start(out=tvec_sb[:], in_=tvec[:])
            nc.sync.dma_start(out=ident_sb[:], in_=ident[:])
            nc.sync.dma_start(out=ta_sb[:], in_=ta[:])
            ones_sb = cst_sb[:, 0:128]
            brow_sb = cst_sb[:, 128 : 128 + NPAD]

            # bulk matrix data on the idle Pool queue, pieces aligned to the
            # emission order of the psum groups
            g8h_sb = const.tile([128, NCH * 4, 512], FP8)
            g8l_sb = const.tile([128, eqw * 4, 512], FP8)
            for s, e in ((15, 18), (12, 15), (4, 8), (8, 12), (0, 4)):
                nc.gpsimd.dma_start(
                    out=g8h_sb[:, 4 * s : 4 * e, :],
                    in_=g8h[:, 2048 * s : 2048 * e],
                )
            nc.gpsimd.dma_start(out=g8l_sb[:], in_=g8l[:])

            outsb = stats.tile([128, 4 * MT], F32)
            a_slot = [stats.tile([128, NG], F32, name=f"a{m}") for m in range(MT)]
            s2_slot = [stats.tile([128, eqw], F32, name=f"s2{m}") for m in range(MT)]
            s3_slot = [stats.tile([128, eqw], F32, name=f"s3{m}") for m in range(MT)]

            for m in range(MT):
                for gi in emit_order:
                    chunks = groups[gi]
                    gw = 512 * len(chunks)
                    ps = psp.tile([128, 2048], F32)
                    for ci, ch in enumerate(chunks):
                        csl = ps[:, 512 * ci : 512 * (ci + 1)]
                        gcs = slice(512 * ch, 512 * (ch + 1))
                        is_win = ch < eqw
                        nc.tensor.matmul(
                            csl, ones_sb, brow_sb[:, gcs], start=True, stop=False
                        )
                        for kp in range(2):
                            nc.tensor.matmul(
                                csl,
                                f8h_sb[:, 2 * kp : 2 * kp + 2, 128 * m : 128 * (m + 1)],
                                g8h_sb[:, 4 * ch + 2 * kp : 4 * ch + 2 * kp + 2, :],
                                start=False,
                                stop=(not is_win and kp == 1),
                                perf_mode=DR,
                            )
                        if is_win:
                            # lo correction terms, narrowed to the match window
                            if ch == 0 and woff is not None:
                                off, n = woff[m], 256
                            elif ch == eqw - 1 and corr_last is not None:
                                off, n = 0, corr_last
                            else:
                                off, n = 0, 512
                            for ti, (lh, rh) in enumerate(
                                ((f8l_sb, g8h_sb), (f8h_sb, g8l_sb))
                            ):
                                for kp in range(2):
                                    nc.tensor.matmul(
                                        ps[:, 512 * ci + off : 512 * ci + off + n],
                                        lh[:, 2 * kp : 2 * kp + 2,
                                           128 * m : 128 * (m + 1)],
                                        rh[:, 4 * ch + 2 * kp : 4 * ch + 2 * kp + 2,
                                           off : off + n],
                                        start=False,
                                        stop=(ti == 1 and kp == 1),
                                        perf_mode=DR,
                                    )
                    # S3/diag only read psum: emit them before the exp pass so
                    # DVE runs concurrently with ACT and the psum buffer is
                    # released the moment ACT finishes.
                    for ci, ch in enumerate(chunks):
                        if ch >= eqw:
                            continue
                        esl = slice(512 * ci, 512 * (ci + 1))
                        tsl = slice(512 * ch, 512 * (ch + 1))
                        sc3 = scr.tile([128, 512], BF16, tag="sc3")
                        nc.vector.scalar_tensor_tensor(
                            out=sc3[:],
                            in0=ta_sb[:, tsl],
                            scalar=tvec_sb[:, m : m + 1],
                            in1=ps[:, esl],
                            op0=ALU.is_equal,
                            op1=ALU.mult,
                            accum_out=s3_slot[m][:, ch : ch + 1],
                        )
                        if ch == 0:
                            # local row p's own column is chunk-0 column
                            # 128m+p: the psum diagonal of this [128,128]
                            # sub-block is the self dot-product (plus bias).
                            sd = scr.tile([128, 128], BF16, tag="sd")
                            nc.vector.scalar_tensor_tensor(
                                out=sd[:],
                                in0=ident_sb[:],
                                scalar=1.0,
                                in1=ps[:, 128 * m : 128 * (m + 1)],
                                op0=ALU.mult,
                                op1=ALU.mult,
                                accum_out=outsb[:, 4 * m + 3 : 4 * m + 4],
                            )
                    e1 = e1p.tile([128, 2048], BF16, name="e1", tag="e1")
                    nc.scalar.activation(
                        e1[:, 0:gw],
                        ps[:, 0:gw],
                        AF.Exp,
                        scale=10.0 / PS,
                        accum_out=a_slot[m][:, gi : gi + 1],
                    )
                    for ci, ch in enumerate(chunks):
                        if ch >= eqw:
                            continue
                        esl = slice(512 * ci, 512 * (ci + 1))
                        tsl = slice(512 * ch, 512 * (ch + 1))
                        sc2 = scr.tile([128, 512], BF16, tag="sc2")
                        nc.vector.scalar_tensor_tensor(
                            out=sc2[:],
                            in0=ta_sb[:, tsl],
                            scalar=tvec_sb[:, m : m + 1],
                            in1=e1[:, esl],
                            op0=ALU.is_equal,
                            op1=ALU.mult,
                            accum_out=s2_slot[m][:, ch : ch + 1],
                        )

                nc.vector.tensor_reduce(
                    outsb[:, 4 * m : 4 * m + 1],
                    a_slot[m][:],
                    mybir.AxisListType.X,
                    ALU.add,
                )
                nc.vector.tensor_reduce(
                    outsb[:, 4 * m + 1 : 4 * m + 2],
                    s2_slot[m][:],
                    mybir.AxisListType.X,
                    ALU.add,
                )
                nc.vector.tensor_reduce(
                    outsb[:, 4 * m + 2 : 4 * m + 3],
                    s3_slot[m][:],
                    mybir.AxisListType.X,
                    ALU.add,
                )
            nc.sync.dma_start(out=out[:], in_=outsb[:])
    _split_multi_waits(nc)
    return nc


_nc_by_cfg = {}


def _get_nc(eqw, corr_last, woff):
    key = (eqw, corr_last, woff)
    if key not in _nc_by_cfg:
        _nc_by_cfg[key] = _build_nc(eqw, corr_last, woff)
    return _nc_by_cfg[key]


def _q8(x):
    return np.clip(np.asarray(x, np.float32), -240.0, 240.0).astype(F8NP)


def _prepare(centers1, features, targets, features_ood, pseudo_target_ood):
    """Host-side O(N log N) prep.

    Rows are globally sorted by class and sharded contiguously, so each
    core's 512 rows cover ~C/8 classes whose other members mostly live in
    the same core. Per core the g columns are permuted to
    [own 512 rows | all other same-class batch cols + own-class centers |
     rest bc cols | ood | pad], which confines every eq-match (and the
    diagonal, at column p for local row p) to the first EQW chunks. Only
    those chunks need the 3-term fp8 split and the masked S2/S3 reductions.
    """
    centers1 = np.asarray(centers1, np.float32)
    features = np.asarray(features, np.float32)
    features_ood = np.asarray(features_ood, np.float32)
    targets = np.asarray(targets).astype(np.int64)
    pseudo = np.asarray(pseudo_target_ood).astype(np.int64)

    tac = np.concatenate([targets, np.arange(C), pseudo])
    w_full = np.bincount(tac, minlength=C).astype(np.float64)

    # class-id label per g row (incl. centers/ood), and bias per g row
    lab = np.concatenate([targets, np.arange(C), np.full(BO, C, np.int64),
                          np.full(PAD, -1, np.int64)])
    bias1 = np.full(NPAD, -20.0, np.float64)
    bias1[:N] = -(np.log(w_full[tac]) + 10.0) / 10.0
    bs = PS * bias1
    b_h = bs.astype(BFNP)
    b_l = (bs - b_h.astype(np.float64)).astype(BFNP)

    g = np.concatenate(
        [features, centers1, features_ood, np.zeros((PAD, D), np.float32)], axis=0
    )
    g_h8 = _q8(SF * g)
    g_l8 = _q8(SF * g - g_h8.astype(np.float32))

    row_perm = np.argsort(targets, kind="stable")
    t_sorted = targets[row_perm]

    ident = np.eye(128, dtype=np.float32)
    ones2 = np.ones((2, 128), BFNP)

    # per-core column permutations
    perms = []
    eqw_need = 1
    mm_max = 0
    all_batch = np.arange(B)
    for c in range(NCORES):
        own = row_perm[RPC * c : RPC * (c + 1)]            # sorted by class
        tset = np.zeros(C + 1, bool)
        tset[t_sorted[RPC * c : RPC * (c + 1)]] = True
        in_own = np.zeros(B, bool)
        in_own[own] = True
        match_b = all_batch[tset[targets] & ~in_own]       # other cores' rows, own classes
        match_c = B + np.flatnonzero(tset[:C])             # centers of own classes
        matched = np.concatenate([match_b, match_c])
        rest_mask = np.ones(B + C, bool)
        rest_mask[own] = False
        rest_mask[matched] = False
        rest = np.flatnonzero(rest_mask)
        perm = np.concatenate(
            [own, matched, rest,
             np.arange(B + C, N),                          # ood
             np.arange(N, NPAD)]                           # pad
        )
        assert perm.shape == (NPAD,)
        perms.append(perm)
        eqw_need = max(eqw_need, -(-(RPC + len(matched)) // 512))
        mm_max = max(mm_max, RPC + len(matched))

    eqw = max(eqw_need, 2)  # chunks that must carry matches (expected 2)
    # columns the last window chunk must cover at full precision
    rem = mm_max - 512 * (eqw - 1)
    corr_last = 256 if rem <= 256 else None

    # chunk-0 correction windows per row-tile: row-tile m only matches own
    # columns whose classes occur in its rows — a narrow band around 128*m.
    WOFF = (0, 64, 192, 256)
    woff = WOFF
    for c in range(NCORES):
        tc_ = t_sorted[RPC * c : RPC * (c + 1)]
        for m in range(MT):
            cmin, cmax = tc_[128 * m], tc_[128 * m + 127]
            lo = np.searchsorted(tc_, cmin, side="left")
            hi = np.searchsorted(tc_, cmax, side="right")
            if not (WOFF[m] <= lo and hi <= WOFF[m] + 256):
                woff = None

    def tile_T(x):
        # [ncols, D] -> [128, (ncols/512)*2048] in the SBUF chunk layout:
        # block ch at ch*2048, inner offset 512*k + j  (k = dim-slice, j = col)
        nch = x.shape[0] // 512
        xt = np.ascontiguousarray(x.T)                     # [D, ncols]
        return np.ascontiguousarray(
            xt.reshape(4, 128, nch, 512).transpose(1, 2, 0, 3).reshape(128, nch * 2048)
        )

    in_maps = []
    for c in range(NCORES):
        perm = perms[c]
        cst_c = np.ascontiguousarray(
            np.concatenate([ones2, np.stack([b_h[perm], b_l[perm]])], axis=1)
        )
        ta_p = lab[perm[: eqw * 512]].astype(np.float32)
        ta_bc = np.ascontiguousarray(np.broadcast_to(ta_p, (128, eqw * 512)))
        tvec_c = np.ascontiguousarray(
            t_sorted[RPC * c : RPC * (c + 1)].reshape(MT, 128).T.astype(np.float32)
        )
        in_maps.append(
            {
                "g8h": tile_T(g_h8[perm]),
                "g8l": tile_T(g_l8[perm[: eqw * 512]]),
                "f8h": tile_T(g_h8[perm[:RPC]]),
                "f8l": tile_T(g_l8[perm[:RPC]]),
                "cst": cst_c,
                "ta": ta_bc,
                "tvec": tvec_c,
                "ident": ident,
            }
        )

    # effective per-class bias as the device psum sees it (fp32 add of pair)
    cls_bias = PS * -(np.log(w_full) + 10.0) / 10.0
    cb_h = cls_bias.astype(BFNP)
    cb_l = (cls_bias - cb_h.astype(np.float64)).astype(BFNP)
    bias_eff_cls = (cb_h.astype(np.float64) + cb_l.astype(np.float64)) / PS

    host = {"t_sorted": t_sorted, "w_full": w_full, "bias_eff_cls": bias_eff_cls,
            "eqw": eqw, "corr_last": corr_last, "woff": woff}
    return in_maps, host


def _combine(results, host):
    t_sorted = host["t_sorted"]
    w_full = host["w_full"]
    cnt_batch = np.bincount(t_sorted, minlength=C).astype(np.float64)

    A = np.empty(B)
    S2 = np.empty(B)
    S3 = np.empty(B)
    diag = np.empty(B)
    for c in range(NCORES):
        o = np.asarray(results[c]["out"], np.float64)  # [128, 16]
        for m in range(MT):
            rs = slice(RPC * c + 128 * m, RPC * c + 128 * (m + 1))
            A[rs] = o[:, 4 * m]
            S2[rs] = o[:, 4 * m + 1]
            S3[rs] = o[:, 4 * m + 2] / PS
            diag[rs] = o[:, 4 * m + 3] / PS

    ws = w_full[t_sorted]
    K = cnt_batch[t_sorted]
    ds_ = 1.0 / (ws - 1.0) - 1.0 / ws
    b1s = host["bias_eff_cls"][t_sorted]
    e1s = np.exp(10.0 * diag)
    S = A - e1s + ds_ * ws * (S2 - e1s)
    P = 10.0 * (S3 - K * b1s - diag) - 10.0 * K
    val = P / K - np.log(S)
    return np.float32(-val.mean())


def _run(inputs, trace=False, **kw):
    in_maps, host = _prepare(**inputs)
    nc = _get_nc(host["eqw"], host["corr_last"], host["woff"])
    res = run_bass_kernel_spmd(nc, in_maps, list(range(NCORES)), trace=trace, **kw)
    loss = _combine(res.results, host)
    return loss, res


def kernel(**inputs):
    loss, _ = _run(inputs)
    return loss


# revision 9
# speedup vs baseline: 1.0585x; 1.0585x over previous
"""Trainium2 Bass kernel for the class-balanced supervised-contrastive loss.

Math (reference semantics, shift-invariant form with constant shift 10):
  l_ij = (f_i . g_j) / T,  T = 0.1, g = [features; centers; features_ood]
  E_ij = exp(l_ij - 10)
  S_i  = sum_{j != i} E_ij / (w_j - eq_ij)        (w_j = class count, eq = label match)
  P_i  = sum_{j != i} eq_ij (l_ij - 10)
  loss = -mean_i( P_i / K_i - log S_i ),  K_i = batch count of class t_i

Key identity: for matched columns j (eq_ij = 1) the class equals t_i, so w_j is a
per-row constant w*. The device therefore only needs, per row:
  A_i  = sum_j exp(10*(r_ij + bias1_j))      bias1_j = (ln(1/w_j) - 10)/10
  S2_i = sum_j eq_ij * E1_ij                 (E1 = the summand of A)
  S3_i = sum_j eq_ij * psum_ij               (psum = 1024*(r + bias1))
  diag = psum_ii                             (self column, for exclusion terms)
Everything else is O(B) host math.

Matmuls run in fp8 e4m3 DoubleRow mode (0.5 PE cycles/col, two K-slices per
instruction). Operands are scaled by 32 on the host, so psum = 1024*r and the
ACT exp uses scale 10/1024. Columns outside the matched window use a single
fh.gh term (random ~2e-2 error per exp term, averages out over ~9k summed
terms); the matched window gets fh.gh + fl.gh + fh.gl (logit error ~7e-4),
with the lo terms narrowed to the 256 columns where matches live. The
per-column bias enters as a K=2 bf16 matmul of ones against the hi/lo pair of
1024*bias1 (error ~1e-5). End-to-end numpy emulation: rel err 2.3e-5.

Device layout per core (rows sharded, 512 rows/core, 4 row tiles):
  per row tile: 5 psum groups of <=4 column chunks ([128,2048] = 4 banks,
  2-buffer ping-pong fills all 8 banks). Group 0 holds the window chunks.
  ACT: E1 = exp(psum*10/1024) with accum_out -> A partial per group
  DVE: scalar_tensor_tensor (ta == t_i) * {E1, psum} -> S2/S3; ident*psum diag
  Bulk g-matrix DMAs ride the otherwise-idle Pool (gpsimd) queue.
"""

import ml_dtypes
import numpy as np

import concourse.bass as bass
import concourse.mybir as mybir
import concourse.tile as tile
from concourse.bass_utils import run_bass_kernel_spmd

NCORES = 8
C, TEMP = 1000, 0.1
B, BO, D = 4096, 4096, 512
N = B + C + BO              # 9192
NPAD = 9216                 # 18 * 512
PAD = NPAD - N
NCH = NPAD // 512           # 18 column chunks
RPC = B // NCORES           # 512 rows per core
MT = RPC // 128             # 4 row tiles per core
SF = 32.0                   # fp8 operand scale; psum = SF*SF*(r + bias1)
PS = SF * SF                # 1024

F32 = mybir.dt.float32
BF16 = mybir.dt.bfloat16
FP8 = mybir.dt.float8e4
ALU = mybir.AluOpType
AF = mybir.ActivationFunctionType
BFNP = ml_dtypes.bfloat16
F8NP = ml_dtypes.float8_e4m3   # TRN FP8_EXP4: max +-240, then inf
DR = mybir.MatmulPerfMode.DoubleRow

# This walrus build accepts only one sync-wait command per engine instruction.
# Move surplus waits onto standalone EventSemaphore instructions just before
# the affected instruction (same engine, so blocking semantics are identical).
_SPLIT_SKIP = ("InstEventSemaphore",)


def _split_multi_waits(nc):
    n = 0
    for f in nc.m.functions:
        for bb in f.blocks:
            new = []
            for ins in bb.instructions:
                si = ins.sync_info
                if (
                    si is not None
                    and si.on_wait
                    and len(si.on_wait) > 1
                    and type(ins).__name__ not in _SPLIT_SKIP
                ):
                    waits = list(si.on_wait)
                    for w in waits[:-1]:
                        es = mybir.InstEventSemaphore(
                            name=f"wsplit_{n}",
                            engine=ins.engine,
                            sync_info=mybir.SyncInfo(on_wait=[w], on_update=[]),
                        )
                        n += 1
                        new.append(es)
                    ins.sync_info = mybir.SyncInfo(
                        on_wait=[waits[-1]], on_update=list(si.on_update)
                    )
                new.append(ins)
            bb.instructions = new
    return n


def _build_nc(eqw=2, corr_last=None, woff=None):
    assert eqw <= 4, "window chunks must fit in psum group 0"
    nc = bass.Bass()
    # host pre-tiles to the SBUF layout: col chunk ch lives at [128, 2048]
    # block ch with inner offset 512*k + j  (k = contraction slice)
    g8h = nc.declare_dram_parameter("g8h", [128, NCH * 2048], FP8, isOutput=False)
    g8l = nc.declare_dram_parameter("g8l", [128, eqw * 2048], FP8, isOutput=False)
    f8h = nc.declare_dram_parameter("f8h", [128, 2048], FP8, isOutput=False)
    f8l = nc.declare_dram_parameter("f8l", [128, 2048], FP8, isOutput=False)
    # [2, .]: row 0 = (ones128, hi of 1024*bias1), row 1 = (ones128, lo part)
    cst = nc.declare_dram_parameter("cst", [2, 128 + NPAD], BF16, isOutput=False)
    ta = nc.declare_dram_parameter("ta", [128, eqw * 512], F32, isOutput=False)
    tvec = nc.declare_dram_parameter("tvec", [128, MT], F32, isOutput=False)
    ident = nc.declare_dram_parameter("ident", [128, 128], F32, isOutput=False)
    out = nc.declare_dram_parameter("out", [128, 4 * MT], F32, isOutput=True)

    # column chunks packed into psum groups sized [4,4,4,3,3]: every ACT
    # drain is then >=1652ns, which covers the PE fill of the group two
    # slots ahead (2-buffer PSUM ping-pong never stalls the pipeline).
    # Emission order per row tile puts the window group (expensive fill,
    # most input tensors) last and the tail chunks (smallest DMA) first.
    groups = [[0, 1, 2, 3], [4, 5, 6, 7], [8, 9, 10, 11],
              [12, 13, 14], [15, 16, 17]]
    emit_order = [4, 3, 1, 2, 0]
    NG = len(groups)

    with tile.TileContext(nc) as tc:
        with (
            tc.tile_pool(name="const", bufs=1) as const,
            tc.tile_pool(name="stats", bufs=1) as stats,
            tc.tile_pool(name="e1", bufs=3) as e1p,
            tc.tile_pool(name="scr", bufs=2) as scr,
            tc.tile_pool(name="psum", bufs=2, space="PSUM") as psp,
        ):
            # preload the exp spline table while DMAs stream in
            warm = const.tile([128, 1], F32)
            nc.vector.memset(warm[:], 0.0)
            nc.scalar.activation(warm[:], warm[:], AF.Exp)

            f8h_sb = const.tile([128, 4, 512], FP8)
            f8l_sb = const.tile([128, 4, 512], FP8)
            cst_sb = const.tile([2, 128 + NPAD], BF16)
            # SP queue, in consumption order: f rows, then bias columns for
            # the first-emitted groups (tail chunks), then the rest.
            nc.sync.dma_start(out=f8h_sb[:], in_=f8h[:])
            nc.sync.dma_start(out=cst_sb[:, 0:128], in_=cst[:, 0:128])
            nc.sync.dma_start(
                out=cst_sb[:, 128 + 6144 :], in_=cst[:, 128 + 6144 :]
            )
            nc.sync.dma_start(
                out=cst_sb[:, 128 : 128 + 2048], in_=cst[:, 128 : 128 + 2048]
            )
            ta_sb = const.tile([128, eqw * 512], F32)
            tvec_sb = const.tile([128, MT], F32)
            ident_sb = const.tile([128, 128], F32)
            nc.sync.dma_start(out=f8l_sb[:], in_=f8l[:])
            nc.sync.dma_start(out=tvec_sb[:], in_=tvec[:])
            nc.sync.dma_start(out=ident_sb[:], in_=ident[:])
            nc.sync.dma_start(out=ta_sb[:], in_=ta[:])
            ones_sb = cst_sb[:, 0:128]
            brow_sb = cst_sb[:, 128 : 128 + NPAD]

            # bulk matrix data split over the idle Pool queue and the SP
            # tail, pieces aligned to the emission order of the psum groups
            g8h_sb = const.tile([128, NCH * 4, 512], FP8)
            g8l_sb = const.tile([128, eqw * 4, 512], FP8)
            nc.gpsimd.dma_start(
                out=g8h_sb[:, 60:72, :], in_=g8h[:, 2048 * 15 :]
            )
            nc.gpsimd.dma_start(
                out=cst_sb[:, 128 + 2048 : 128 + 6144],
                in_=cst[:, 128 + 2048 : 128 + 6144],
            )
            for s, e in ((12, 15), (4, 8), (8, 12)):
                nc.gpsimd.dma_start(
                    out=g8h_sb[:, 4 * s : 4 * e, :],
                    in_=g8h[:, 2048 * s : 2048 * e],
                )
            nc.sync.dma_start(out=g8h_sb[:, 0:16, :], in_=g8h[:, 0:8192])
            nc.sync.dma_start(out=g8l_sb[:], in_=g8l[:])

            outsb = stats.tile([128, 4 * MT], F32)
            a_slot = [stats.tile([128, NG], F32, name=f"a{m}") for m in range(MT)]
            s2_slot = [stats.tile([128, eqw], F32, name=f"s2{m}") for m in range(MT)]
            s3_slot = [stats.tile([128, eqw], F32, name=f"s3{m}") for m in range(MT)]

            for m in range(MT):
                for gi in emit_order:
                    chunks = groups[gi]
                    gw = 512 * len(chunks)
                    ps = psp.tile([128, 2048], F32)
                    for ci, ch in enumerate(chunks):
                        csl = ps[:, 512 * ci : 512 * (ci + 1)]
                        gcs = slice(512 * ch, 512 * (ch + 1))
                        is_win = ch < eqw
                        nc.tensor.matmul(
                            csl, ones_sb, brow_sb[:, gcs], start=True, stop=False
                        )
                        for kp in range(2):
                            nc.tensor.matmul(
                                csl,
                                f8h_sb[:, 2 * kp : 2 * kp + 2, 128 * m : 128 * (m + 1)],
                                g8h_sb[:, 4 * ch + 2 * kp : 4 * ch + 2 * kp + 2, :],
                                start=False,
                                stop=(not is_win and kp == 1),
                                perf_mode=DR,
                            )
                        if is_win:
                            # lo correction terms, narrowed to the match window
                            if ch == 0 and woff is not None:
                                off, n = woff[m], 256
                            elif ch == eqw - 1 and corr_last is not None:
                                off, n = 0, corr_last
                            else:
                                off, n = 0, 512
                            for ti, (lh, rh) in enumerate(
                                ((f8l_sb, g8h_sb), (f8h_sb, g8l_sb))
                            ):
                                for kp in range(2):
                                    nc.tensor.matmul(
                                        ps[:, 512 * ci + off : 512 * ci + off + n],
                                        lh[:, 2 * kp : 2 * kp + 2,
                                           128 * m : 128 * (m + 1)],
                                        rh[:, 4 * ch + 2 * kp : 4 * ch + 2 * kp + 2,
                                           off : off + n],
                                        start=False,
                                        stop=(ti == 1 and kp == 1),
                                        perf_mode=DR,
                                    )
                    # PSUM readers from different engines serialize in
                    # emission order, so DVE takes one fp32 snapshot of the
                    # window columns (fits in the previous group's ACT drain)
                    # and every masked reduction reads SBUF afterwards; ACT is
                    # then the sole remaining psum reader and releases the
                    # buffer the moment it finishes.
                    nwin = sum(1 for ch in chunks if ch < eqw)
                    pc = None
                    if nwin:
                        pc = scr.tile([128, 512 * nwin], F32, tag="pc")
                        nc.vector.tensor_copy(
                            out=pc[:], in_=ps[:, 0 : 512 * nwin]
                        )
                    e1 = e1p.tile([128, 2048], BF16, name="e1", tag="e1")
                    nc.scalar.activation(
                        e1[:, 0:gw],
                        ps[:, 0:gw],
                        AF.Exp,
                        scale=10.0 / PS,
                        accum_out=a_slot[m][:, gi : gi + 1],
                    )
                    for ci, ch in enumerate(chunks):
                        if ch >= eqw:
                            continue
                        esl = slice(512 * ci, 512 * (ci + 1))
                        tsl = slice(512 * ch, 512 * (ch + 1))
                        sc3 = scr.tile([128, 512], BF16, tag="sc3")
                        nc.vector.scalar_tensor_tensor(
                            out=sc3[:],
                            in0=ta_sb[:, tsl],
                            scalar=tvec_sb[:, m : m + 1],
                            in1=pc[:, esl],
                            op0=ALU.is_equal,
                            op1=ALU.mult,
                            accum_out=s3_slot[m][:, ch : ch + 1],
                        )
                        if ch == 0:
                            # local row p's own column is chunk-0 column
                            # 128m+p: the psum diagonal of this [128,128]
                            # sub-block is the self dot-product (plus bias).
                            sd = scr.tile([128, 128], BF16, tag="sd")
                            nc.vector.scalar_tensor_tensor(
                                out=sd[:],
                                in0=ident_sb[:],
                                scalar=1.0,
                                in1=pc[:, 128 * m : 128 * (m + 1)],
                                op0=ALU.mult,
                                op1=ALU.mult,
                                accum_out=outsb[:, 4 * m + 3 : 4 * m + 4],
                            )
                        sc2 = scr.tile([128, 512], BF16, tag="sc2")
                        nc.vector.scalar_tensor_tensor(
                            out=sc2[:],
                            in0=ta_sb[:, tsl],
                            scalar=tvec_sb[:, m : m + 1],
                            in1=e1[:, esl],
                            op0=ALU.is_equal,
                            op1=ALU.mult,
                            accum_out=s2_slot[m][:, ch : ch + 1],
                        )

                nc.vector.tensor_reduce(
                    outsb[:, 4 * m : 4 * m + 1],
                    a_slot[m][:],
                    mybir.AxisListType.X,
                    ALU.add,
                )
                nc.vector.tensor_reduce(
                    outsb[:, 4 * m + 1 : 4 * m + 2],
                    s2_slot[m][:],
                    mybir.AxisListType.X,
                    ALU.add,
                )
                nc.vector.tensor_reduce(
                    outsb[:, 4 * m + 2 : 4 * m + 3],
                    s3_slot[m][:],
                    mybir.AxisListType.X,
                    ALU.add,
                )
            nc.sync.dma_start(out=out[:], in_=outsb[:])
    _split_multi_waits(nc)
    return nc


_nc_by_cfg = {}


def _get_nc(eqw, corr_last, woff):
    key = (eqw, corr_last, woff)
    if key not in _nc_by_cfg:
        _nc_by_cfg[key] = _build_nc(eqw, corr_last, woff)
    return _nc_by_cfg[key]


def _q8(x):
    return np.clip(np.asarray(x, np.float32), -240.0, 240.0).astype(F8NP)


def _prepare(centers1, features, targets, features_ood, pseudo_target_ood):
    """Host-side O(N log N) prep.

    Rows are globally sorted by class and sharded contiguously, so each
    core's 512 rows cover ~C/8 classes whose other members mostly live in
    the same core. Per core the g columns are permuted to
    [own 512 rows | all other same-class batch cols + own-class centers |
     rest bc cols | ood | pad], which confines every eq-match (and the
    diagonal, at column p for local row p) to the first EQW chunks. Only
    those chunks need the 3-term fp8 split and the masked S2/S3 reductions.
    """
    centers1 = np.asarray(centers1, np.float32)
    features = np.asarray(features, np.float32)
    features_ood = np.asarray(features_ood, np.float32)
    targets = np.asarray(targets).astype(np.int64)
    pseudo = np.asarray(pseudo_target_ood).astype(np.int64)

    tac = np.concatenate([targets, np.arange(C), pseudo])
    w_full = np.bincount(tac, minlength=C).astype(np.float64)

    # class-id label per g row (incl. centers/ood), and bias per g row
    lab = np.concatenate([targets, np.arange(C), np.full(BO, C, np.int64),
                          np.full(PAD, -1, np.int64)])
    bias1 = np.full(NPAD, -20.0, np.float64)
    bias1[:N] = -(np.log(w_full[tac]) + 10.0) / 10.0
    bs = PS * bias1
    b_h = bs.astype(BFNP)
    b_l = (bs - b_h.astype(np.float64)).astype(BFNP)

    g = np.concatenate(
        [features, centers1, features_ood, np.zeros((PAD, D), np.float32)], axis=0
    )
    g_h8 = _q8(SF * g)
    g_l8 = _q8(SF * g - g_h8.astype(np.float32))

    row_perm = np.argsort(targets, kind="stable")
    t_sorted = targets[row_perm]

    ident = np.eye(128, dtype=np.float32)
    ones2 = np.ones((2, 128), BFNP)

    # per-core column permutations
    perms = []
    eqw_need = 1
    mm_max = 0
    all_batch = np.arange(B)
    for c in range(NCORES):
        own = row_perm[RPC * c : RPC * (c + 1)]            # sorted by class
        tset = np.zeros(C + 1, bool)
        tset[t_sorted[RPC * c : RPC * (c + 1)]] = True
        in_own = np.zeros(B, bool)
        in_own[own] = True
        match_b = all_batch[tset[targets] & ~in_own]       # other cores' rows, own classes
        match_c = B + np.flatnonzero(tset[:C])             # centers of own classes
        matched = np.concatenate([match_b, match_c])
        rest_mask = np.ones(B + C, bool)
        rest_mask[own] = False
        rest_mask[matched] = False
        rest = np.flatnonzero(rest_mask)
        perm = np.concatenate(
            [own, matched, rest,
             np.arange(B + C, N),                          # ood
             np.arange(N, NPAD)]                           # pad
        )
        assert perm.shape == (NPAD,)
        perms.append(perm)
        eqw_need = max(eqw_need, -(-(RPC + len(matched)) // 512))
        mm_max = max(mm_max, RPC + len(matched))

    eqw = max(eqw_need, 2)  # chunks that must carry matches (expected 2)
    # columns the last window chunk must cover at full precision
    rem = mm_max - 512 * (eqw - 1)
    corr_last = 256 if rem <= 256 else None

    # chunk-0 correction windows per row-tile: row-tile m only matches own
    # columns whose classes occur in its rows — a narrow band around 128*m.
    WOFF = (0, 64, 192, 256)
    woff = WOFF
    for c in range(NCORES):
        tc_ = t_sorted[RPC * c : RPC * (c + 1)]
        for m in range(MT):
            cmin, cmax = tc_[128 * m], tc_[128 * m + 127]
            lo = np.searchsorted(tc_, cmin, side="left")
            hi = np.searchsorted(tc_, cmax, side="right")
            if not (WOFF[m] <= lo and hi <= WOFF[m] + 256):
                woff = None

    def tile_T(x):
        # [ncols, D] -> [128, (ncols/512)*2048] in the SBUF chunk layout:
        # block ch at ch*2048, inner offset 512*k + j  (k = dim-slice, j = col)
        nch = x.shape[0] // 512
        xt = np.ascontiguousarray(x.T)                     # [D, ncols]
        return np.ascontiguousarray(
            xt.reshape(4, 128, nch, 512).transpose(1, 2, 0, 3).reshape(128, nch * 2048)
        )

    in_maps = []
    for c in range(NCORES):
        perm = perms[c]
        cst_c = np.ascontiguousarray(
            np.concatenate([ones2, np.stack([b_h[perm], b_l[perm]])], axis=1)
        )
        ta_p = lab[perm[: eqw * 512]].astype(np.float32)
        ta_bc = np.ascontiguousarray(np.broadcast_to(ta_p, (128, eqw * 512)))
        tvec_c = np.ascontiguousarray(
            t_sorted[RPC * c : RPC * (c + 1)].reshape(MT, 128).T.astype(np.float32)
        )
        in_maps.append(
            {
                "g8h": tile_T(g_h8[perm]),
                "g8l": tile_T(g_l8[perm[: eqw * 512]]),
                "f8h": tile_T(g_h8[perm[:RPC]]),
                "f8l": tile_T(g_l8[perm[:RPC]]),
                "cst": cst_c,
                "ta": ta_bc,
                "tvec": tvec_c,
                "ident": ident,
            }
        )

    # effective per-class bias as the device psum sees it (fp32 add of pair)
    cls_bias = PS * -(np.log(w_full) + 10.0) / 10.0
    cb_h = cls_bias.astype(BFNP)
    cb_l = (cls_bias - cb_h.astype(np.float64)).astype(BFNP)
    bias_eff_cls = (cb_h.astype(np.float64) + cb_l.astype(np.float64)) / PS

    host = {"t_sorted": t_sorted, "w_full": w_full, "bias_eff_cls": bias_eff_cls,
            "eqw": eqw, "corr_last": corr_last, "woff": woff}
    return in_maps, host


def _combine(results, host):
    t_sorted = host["t_sorted"]
    w_full = host["w_full"]
    cnt_batch = np.bincount(t_sorted, minlength=C).astype(np.float64)

    A = np.empty(B)
    S2 = np.empty(B)
    S3 = np.empty(B)
    diag = np.empty(B)
    for c in range(NCORES):
        o = np.asarray(results[c]["out"], np.float64)  # [128, 16]
        for m in range(MT):
            rs = slice(RPC * c + 128 * m, RPC * c + 128 * (m + 1))
            A[rs] = o[:, 4 * m]
            S2[rs] = o[:, 4 * m + 1]
            S3[rs] = o[:, 4 * m + 2] / PS
            diag[rs] = o[:, 4 * m + 3] / PS

    ws = w_full[t_sorted]
    K = cnt_batch[t_sorted]
    ds_ = 1.0 / (ws - 1.0) - 1.0 / ws
    b1s = host["bias_eff_cls"][t_sorted]
    e1s = np.exp(10.0 * diag)
    S = A - e1s + ds_ * ws * (S2 - e1s)
    P = 10.0 * (S3 - K * b1s - diag) - 10.0 * K
    val = P / K - np.log(S)
    return np.float32(-val.mean())


def _run(inputs, trace=False, **kw):
    in_maps, host = _prepare(**inputs)
    nc = _get_nc(host["eqw"], host["corr_last"], host["woff"])
    res = run_bass_kernel_spmd(nc, in_maps, list(range(NCORES)), trace=trace, **kw)
    loss = _combine(res.results, host)
    return loss, res


def kernel(**inputs):
    loss, _ = _run(inputs)
    return loss


# revision 11
# speedup vs baseline: 1.1467x; 1.0834x over previous
"""Trainium2 Bass kernel for the class-balanced supervised-contrastive loss.

Math (reference semantics, shift-invariant form with constant shift 10):
  l_ij = (f_i . g_j) / T,  T = 0.1, g = [features; centers; features_ood]
  E_ij = exp(l_ij - 10)
  S_i  = sum_{j != i} E_ij / (w_j - eq_ij)        (w_j = class count, eq = label match)
  P_i  = sum_{j != i} eq_ij (l_ij - 10)
  loss = -mean_i( P_i / K_i - log S_i ),  K_i = batch count of class t_i

Key identity: for matched columns j (eq_ij = 1) the class equals t_i, so w_j is a
per-row constant w*. The device therefore only needs, per row:
  A_i  = sum_j exp(10*(r_ij + bias1_j))      bias1_j = (ln(1/w_j) - 10)/10
  S2_i = sum_j eq_ij * E1_ij                 (E1 = the summand of A)
  S3_i = sum_j eq_ij * psum_ij               (psum = 1024*(r + bias1))
  diag = psum_ii                             (self column, for exclusion terms)
Everything else is O(B) host math.

Matmuls run in fp8 e4m3 DoubleRow mode (0.5 PE cycles/col, two K-slices per
instruction). Operands are scaled by 32 on the host, so psum = 1024*r and the
ACT exp uses scale 10/1024. Columns outside the matched window use a single
fh.gh term (random ~2e-2 error per exp term, averages out over ~9k summed
terms); the matched window gets fh.gh + fl.gh + fh.gl (logit error ~7e-4),
with the lo terms narrowed to the 256 columns where matches live. The
per-column bias enters as a K=2 bf16 matmul of ones against the hi/lo pair of
1024*bias1 (error ~1e-5). End-to-end numpy emulation: rel err 2.3e-5.

Device layout per core (rows sharded, 512 rows/core, 4 row tiles):
  per row tile: 5 psum groups of <=4 column chunks ([128,2048] = 4 banks,
  2-buffer ping-pong fills all 8 banks). Group 0 holds the window chunks.
  ACT: E1 = exp(psum*10/1024) with accum_out -> A partial per group
  DVE: scalar_tensor_tensor (ta == t_i) * {E1, psum} -> S2/S3; ident*psum diag
  Bulk g-matrix DMAs ride the otherwise-idle Pool (gpsimd) queue.
"""

import ml_dtypes
import numpy as np

import concourse.bass as bass
import concourse.mybir as mybir
import concourse.tile as tile
from concourse.bass_utils import run_bass_kernel_spmd

NCORES = 8
C, TEMP = 1000, 0.1
B, BO, D = 4096, 4096, 512
N = B + C + BO              # 9192
NPAD = 9216                 # 18 * 512
PAD = NPAD - N
NCH = NPAD // 512           # 18 column chunks
RPC = B // NCORES           # 512 rows per core
MT = RPC // 128             # 4 row tiles per core
SF = 32.0                   # fp8 operand scale; psum = SF*SF*(r + bias1)
PS = SF * SF                # 1024

F32 = mybir.dt.float32
BF16 = mybir.dt.bfloat16
FP8 = mybir.dt.float8e4
ALU = mybir.AluOpType
AF = mybir.ActivationFunctionType
BFNP = ml_dtypes.bfloat16
F8NP = ml_dtypes.float8_e4m3   # TRN FP8_EXP4: max +-240, then inf
DR = mybir.MatmulPerfMode.DoubleRow

# This walrus build accepts only one sync-wait command per engine instruction.
# Move surplus waits onto standalone EventSemaphore instructions just before
# the affected instruction (same engine, so blocking semantics are identical).
_SPLIT_SKIP = ("InstEventSemaphore",)


def _split_multi_waits(nc):
    n = 0
    for f in nc.m.functions:
        for bb in f.blocks:
            new = []
            for ins in bb.instructions:
                si = ins.sync_info
                if (
                    si is not None
                    and si.on_wait
                    and len(si.on_wait) > 1
                    and type(ins).__name__ not in _SPLIT_SKIP
                ):
                    waits = list(si.on_wait)
                    for w in waits[:-1]:
                        es = mybir.InstEventSemaphore(
                            name=f"wsplit_{n}",
                            engine=ins.engine,
                            sync_info=mybir.SyncInfo(on_wait=[w], on_update=[]),
                        )
                        n += 1
                        new.append(es)
                    ins.sync_info = mybir.SyncInfo(
                        on_wait=[waits[-1]], on_update=list(si.on_update)
                    )
                new.append(ins)
            bb.instructions = new
    return n


def _build_nc(eqw=2, corr_last=None, woff=None):
    assert eqw <= 4, "window chunks must fit in psum group 0"
    nc = bass.Bass()
    # host pre-tiles to the SBUF layout: col chunk ch lives at [128, 2048]
    # block ch with inner offset 512*k + j  (k = contraction slice)
    g8h = nc.declare_dram_parameter("g8h", [128, NCH * 2048], FP8, isOutput=False)
    g8l = nc.declare_dram_parameter("g8l", [128, eqw * 2048], FP8, isOutput=False)
    f8h = nc.declare_dram_parameter("f8h", [128, 2048], FP8, isOutput=False)
    f8l = nc.declare_dram_parameter("f8l", [128, 2048], FP8, isOutput=False)
    # [2, .]: row 0 = (ones128, hi of 1024*bias1), row 1 = (ones128, lo part)
    cst = nc.declare_dram_parameter("cst", [2, 128 + NPAD], BF16, isOutput=False)
    ta = nc.declare_dram_parameter("ta", [128, eqw * 512], F32, isOutput=False)
    tvec = nc.declare_dram_parameter("tvec", [128, MT], F32, isOutput=False)
    ident = nc.declare_dram_parameter("ident", [128, 128], F32, isOutput=False)
    out = nc.declare_dram_parameter("out", [128, 4 * MT], F32, isOutput=True)

    # column chunks packed into psum groups sized [4,4,4,3,3]: every ACT
    # drain is then >=1652ns, which covers the PE fill of the group two
    # slots ahead (2-buffer PSUM ping-pong never stalls the pipeline).
    # The two window chunks lead separate groups so each group's fill +
    # window-psum snapshot chain stays under one ACT drain. Emission order
    # per row tile puts the window groups (most input tensors) last and
    # the tail chunks (own DMA piece, smallest bias range) first.
    groups = [[0, 2, 3, 4], [1, 5, 6, 7], [8, 9, 10, 11],
              [12, 13, 14], [15, 16, 17]]
    emit_order = [4, 3, 2, 0, 1]
    NG = len(groups)

    with tile.TileContext(nc) as tc:
        with (
            tc.tile_pool(name="const", bufs=1) as const,
            tc.tile_pool(name="stats", bufs=1) as stats,
            tc.tile_pool(name="e1", bufs=3) as e1p,
            tc.tile_pool(name="scr", bufs=2) as scr,
            tc.tile_pool(name="psum", bufs=2, space="PSUM") as psp,
        ):
            # preload the exp spline table while DMAs stream in
            warm = const.tile([128, 1], F32)
            nc.vector.memset(warm[:], 0.0)
            nc.scalar.activation(warm[:], warm[:], AF.Exp)

            f8h_sb = const.tile([128, 4, 512], FP8)
            f8l_sb = const.tile([128, 4, 512], FP8)
            cst_sb = const.tile([2, 128 + NPAD], BF16)
            g8h_sb = const.tile([128, NCH * 4, 512], FP8)
            g8l_sb = const.tile([128, eqw * 4, 512], FP8)
            ta_sb = const.tile([128, eqw * 512], F32)
            tvec_sb = const.tile([128, MT], F32)
            ident_sb = const.tile([128, 128], F32)
            ones_sb = cst_sb[:, 0:128]
            brow_sb = cst_sb[:, 128 : 128 + NPAD]

            # DMA split across the SP, Pool, and (idle-at-start) ACT queues,
            # each in the consumption order of the emitted psum groups.
            nc.sync.dma_start(
                out=cst_sb[:, 128 + 7680 :], in_=cst[:, 128 + 7680 :]
            )
            nc.sync.dma_start(out=f8h_sb[:], in_=f8h[:])
            nc.sync.dma_start(out=cst_sb[:, 0:128], in_=cst[:, 0:128])
            nc.sync.dma_start(
                out=cst_sb[:, 128 + 6144 : 128 + 7680],
                in_=cst[:, 128 + 6144 : 128 + 7680],
            )
            nc.sync.dma_start(
                out=cst_sb[:, 128 + 2048 : 128 + 6144],
                in_=cst[:, 128 + 2048 : 128 + 6144],
            )
            nc.sync.dma_start(
                out=cst_sb[:, 128 : 128 + 2048], in_=cst[:, 128 : 128 + 2048]
            )
            nc.sync.dma_start(out=f8l_sb[:], in_=f8l[:])
            nc.sync.dma_start(out=tvec_sb[:], in_=tvec[:])
            nc.sync.dma_start(out=ident_sb[:], in_=ident[:])
            nc.sync.dma_start(out=ta_sb[:], in_=ta[:])
            nc.sync.dma_start(out=g8l_sb[:], in_=g8l[:])

            for s, e in ((15, 18), (12, 15), (8, 12), (4, 8)):
                nc.gpsimd.dma_start(
                    out=g8h_sb[:, 4 * s : 4 * e, :],
                    in_=g8h[:, 2048 * s : 2048 * e],
                )
            nc.scalar.dma_start(out=g8h_sb[:, 0:16, :], in_=g8h[:, 0:8192])

            outsb = stats.tile([128, 4 * MT], F32)
            a_slot = [stats.tile([128, NG], F32, name=f"a{m}") for m in range(MT)]
            s2_slot = [stats.tile([128, eqw], F32, name=f"s2{m}") for m in range(MT)]
            s3_slot = [stats.tile([128, eqw], F32, name=f"s3{m}") for m in range(MT)]

            for m in range(MT):
                for gi in emit_order:
                    chunks = groups[gi]
                    gw = 512 * len(chunks)
                    ps = psp.tile([128, 2048], F32)
                    for ci, ch in enumerate(chunks):
                        csl = ps[:, 512 * ci : 512 * (ci + 1)]
                        gcs = slice(512 * ch, 512 * (ch + 1))
                        is_win = ch < eqw
                        nc.tensor.matmul(
                            csl, ones_sb, brow_sb[:, gcs], start=True, stop=False
                        )
                        for kp in range(2):
                            nc.tensor.matmul(
                                csl,
                                f8h_sb[:, 2 * kp : 2 * kp + 2, 128 * m : 128 * (m + 1)],
                                g8h_sb[:, 4 * ch + 2 * kp : 4 * ch + 2 * kp + 2, :],
                                start=False,
                                stop=(not is_win and kp == 1),
                                perf_mode=DR,
                            )
                        if is_win:
                            # lo correction terms, narrowed to the match window
                            if ch == 0 and woff is not None:
                                off, n = woff[m], 256
                            elif ch == eqw - 1 and corr_last is not None:
                                off, n = 0, corr_last
                            else:
                                off, n = 0, 512
                            for ti, (lh, rh) in enumerate(
                                ((f8l_sb, g8h_sb), (f8h_sb, g8l_sb))
                            ):
                                for kp in range(2):
                                    nc.tensor.matmul(
                                        ps[:, 512 * ci + off : 512 * ci + off + n],
                                        lh[:, 2 * kp : 2 * kp + 2,
                                           128 * m : 128 * (m + 1)],
                                        rh[:, 4 * ch + 2 * kp : 4 * ch + 2 * kp + 2,
                                           off : off + n],
                                        start=False,
                                        stop=(ti == 1 and kp == 1),
                                        perf_mode=DR,
                                    )
                    # PSUM readers from different engines serialize in
                    # emission order, so DVE takes one fp32 snapshot of the
                    # window columns (fits in the previous group's ACT drain)
                    # and every masked reduction reads SBUF afterwards; ACT is
                    # then the sole remaining psum reader and releases the
                    # buffer the moment it finishes.
                    nwin = sum(1 for ch in chunks if ch < eqw)
                    pc = None
                    if nwin:
                        pc = scr.tile([128, 512 * nwin], F32, tag="pc")
                        nc.vector.tensor_copy(
                            out=pc[:], in_=ps[:, 0 : 512 * nwin]
                        )
                    e1 = e1p.tile([128, 2048], BF16, name="e1", tag="e1")
                    nc.scalar.activation(
                        e1[:, 0:gw],
                        ps[:, 0:gw],
                        AF.Exp,
                        scale=10.0 / PS,
                        accum_out=a_slot[m][:, gi : gi + 1],
                    )
                    for ci, ch in enumerate(chunks):
                        if ch >= eqw:
                            continue
                        esl = slice(512 * ci, 512 * (ci + 1))
                        tsl = slice(512 * ch, 512 * (ch + 1))
                        sc3 = scr.tile([128, 512], BF16, tag="sc3")
                        nc.vector.scalar_tensor_tensor(
                            out=sc3[:],
                            in0=ta_sb[:, tsl],
                            scalar=tvec_sb[:, m : m + 1],
                            in1=pc[:, esl],
                            op0=ALU.is_equal,
                            op1=ALU.mult,
                            accum_out=s3_slot[m][:, ch : ch + 1],
                        )
                        if ch == 0:
                            # local row p's own column is chunk-0 column
                            # 128m+p: the psum diagonal of this [128,128]
                            # sub-block is the self dot-product (plus bias).
                            sd = scr.tile([128, 128], BF16, tag="sd")
                            nc.vector.scalar_tensor_tensor(
                                out=sd[:],
                                in0=ident_sb[:],
                                scalar=1.0,
                                in1=pc[:, 128 * m : 128 * (m + 1)],
                                op0=ALU.mult,
                                op1=ALU.mult,
                                accum_out=outsb[:, 4 * m + 3 : 4 * m + 4],
                            )
                        sc2 = scr.tile([128, 512], BF16, tag="sc2")
                        nc.vector.scalar_tensor_tensor(
                            out=sc2[:],
                            in0=ta_sb[:, tsl],
                            scalar=tvec_sb[:, m : m + 1],
                            in1=e1[:, esl],
                            op0=ALU.is_equal,
                            op1=ALU.mult,
                            accum_out=s2_slot[m][:, ch : ch + 1],
                        )

                nc.vector.tensor_reduce(
                    outsb[:, 4 * m : 4 * m + 1],
                    a_slot[m][:],
                    mybir.AxisListType.X,
                    ALU.add,
                )
                nc.vector.tensor_reduce(
                    outsb[:, 4 * m + 1 : 4 * m + 2],
                    s2_slot[m][:],
                    mybir.AxisListType.X,
                    ALU.add,
                )
                nc.vector.tensor_reduce(
                    outsb[:, 4 * m + 2 : 4 * m + 3],
                    s3_slot[m][:],
                    mybir.AxisListType.X,
                    ALU.add,
                )
            nc.sync.dma_start(out=out[:], in_=outsb[:])
    _split_multi_waits(nc)
    return nc


_nc_by_cfg = {}


def _get_nc(eqw, corr_last, woff):
    key = (eqw, corr_last, woff)
    if key not in _nc_by_cfg:
        _nc_by_cfg[key] = _build_nc(eqw, corr_last, woff)
    return _nc_by_cfg[key]


def _q8(x):
    return np.clip(np.asarray(x, np.float32), -240.0, 240.0).astype(F8NP)


def _prepare(centers1, features, targets, features_ood, pseudo_target_ood):
    """Host-side O(N log N) prep.

    Rows are globally sorted by class and sharded contiguously, so each
    core's 512 rows cover ~C/8 classes whose other members mostly live in
    the same core. Per core the g columns are permuted to
    [own 512 rows | all other same-class batch cols + own-class centers |
     rest bc cols | ood | pad], which confines every eq-match (and the
    diagonal, at column p for local row p) to the first EQW chunks. Only
    those chunks need the 3-term fp8 split and the masked S2/S3 reductions.
    """
    centers1 = np.asarray(centers1, np.float32)
    features = np.asarray(features, np.float32)
    features_ood = np.asarray(features_ood, np.float32)
    targets = np.asarray(targets).astype(np.int64)
    pseudo = np.asarray(pseudo_target_ood).astype(np.int64)

    tac = np.concatenate([targets, np.arange(C), pseudo])
    w_full = np.bincount(tac, minlength=C).astype(np.float64)

    # class-id label per g row (incl. centers/ood), and bias per g row
    lab = np.concatenate([targets, np.arange(C), np.full(BO, C, np.int64),
                          np.full(PAD, -1, np.int64)])
    bias1 = np.full(NPAD, -20.0, np.float64)
    bias1[:N] = -(np.log(w_full[tac]) + 10.0) / 10.0
    bs = PS * bias1
    b_h = bs.astype(BFNP)
    b_l = (bs - b_h.astype(np.float64)).astype(BFNP)

    g = np.concatenate(
        [features, centers1, features_ood, np.zeros((PAD, D), np.float32)], axis=0
    )
    g_h8 = _q8(SF * g)
    g_l8 = _q8(SF * g - g_h8.astype(np.float32))

    row_perm = np.argsort(targets, kind="stable")
    t_sorted = targets[row_perm]

    ident = np.eye(128, dtype=np.float32)
    ones2 = np.ones((2, 128), BFNP)

    # per-core column permutations
    perms = []
    eqw_need = 1
    mm_max = 0
    all_batch = np.arange(B)
    for c in range(NCORES):
        own = row_perm[RPC * c : RPC * (c + 1)]            # sorted by class
        tset = np.zeros(C + 1, bool)
        tset[t_sorted[RPC * c : RPC * (c + 1)]] = True
        in_own = np.zeros(B, bool)
        in_own[own] = True
        match_b = all_batch[tset[targets] & ~in_own]       # other cores' rows, own classes
        match_c = B + np.flatnonzero(tset[:C])             # centers of own classes
        matched = np.concatenate([match_b, match_c])
        rest_mask = np.ones(B + C, bool)
        rest_mask[own] = False
        rest_mask[matched] = False
        rest = np.flatnonzero(rest_mask)
        perm = np.concatenate(
            [own, matched, rest,
             np.arange(B + C, N),                          # ood
             np.arange(N, NPAD)]                           # pad
        )
        assert perm.shape == (NPAD,)
        perms.append(perm)
        eqw_need = max(eqw_need, -(-(RPC + len(matched)) // 512))
        mm_max = max(mm_max, RPC + len(matched))

    eqw = max(eqw_need, 2)  # chunks that must carry matches (expected 2)
    # columns the last window chunk must cover at full precision
    rem = mm_max - 512 * (eqw - 1)
    corr_last = 256 if rem <= 256 else None

    # chunk-0 correction windows per row-tile: row-tile m only matches own
    # columns whose classes occur in its rows — a narrow band around 128*m.
    WOFF = (0, 64, 192, 256)
    woff = WOFF
    for c in range(NCORES):
        tc_ = t_sorted[RPC * c : RPC * (c + 1)]
        for m in range(MT):
            cmin, cmax = tc_[128 * m], tc_[128 * m + 127]
            lo = np.searchsorted(tc_, cmin, side="left")
            hi = np.searchsorted(tc_, cmax, side="right")
            if not (WOFF[m] <= lo and hi <= WOFF[m] + 256):
                woff = None

    def tile_T(x):
        # [ncols, D] -> [128, (ncols/512)*2048] in the SBUF chunk layout:
        # block ch at ch*2048, inner offset 512*k + j  (k = dim-slice, j = col)
        nch = x.shape[0] // 512
        xt = np.ascontiguousarray(x.T)                     # [D, ncols]
        return np.ascontiguousarray(
            xt.reshape(4, 128, nch, 512).transpose(1, 2, 0, 3).reshape(128, nch * 2048)
        )

    in_maps = []
    for c in range(NCORES):
        perm = perms[c]
        cst_c = np.ascontiguousarray(
            np.concatenate([ones2, np.stack([b_h[perm], b_l[perm]])], axis=1)
        )
        ta_p = lab[perm[: eqw * 512]].astype(np.float32)
        ta_bc = np.ascontiguousarray(np.broadcast_to(ta_p, (128, eqw * 512)))
        tvec_c = np.ascontiguousarray(
            t_sorted[RPC * c : RPC * (c + 1)].reshape(MT, 128).T.astype(np.float32)
        )
        in_maps.append(
            {
                "g8h": tile_T(g_h8[perm]),
                "g8l": tile_T(g_l8[perm[: eqw * 512]]),
                "f8h": tile_T(g_h8[perm[:RPC]]),
                "f8l": tile_T(g_l8[perm[:RPC]]),
                "cst": cst_c,
                "ta": ta_bc,
                "tvec": tvec_c,
                "ident": ident,
            }
        )

    # effective per-class bias as the device psum sees it (fp32 add of pair)
    cls_bias = PS * -(np.log(w_full) + 10.0) / 10.0
    cb_h = cls_bias.astype(BFNP)
    cb_l = (cls_bias - cb_h.astype(np.float64)).astype(BFNP)
    bias_eff_cls = (cb_h.astype(np.float64) + cb_l.astype(np.float64)) / PS

    host = {"t_sorted": t_sorted, "w_full": w_full, "bias_eff_cls": bias_eff_cls,
            "eqw": eqw, "corr_last": corr_last, "woff": woff}
    return in_maps, host


def _combine(results, host):
    t_sorted = host["t_sorted"]
    w_full = host["w_full"]
    cnt_batch = np.bincount(t_sorted, minlength=C).astype(np.float64)

    A = np.empty(B)
    S2 = np.empty(B)
    S3 = np.empty(B)
    diag = np.empty(B)
    for c in range(NCORES):
        o = np.asarray(results[c]["out"], np.float64)  # [128, 16]
        for m in range(MT):
            rs = slice(RPC * c + 128 * m, RPC * c + 128 * (m + 1))
            A[rs] = o[:, 4 * m]
            S2[rs] = o[:, 4 * m + 1]
            S3[rs] = o[:, 4 * m + 2] / PS
            diag[rs] = o[:, 4 * m + 3] / PS

    ws = w_full[t_sorted]
    K = cnt_batch[t_sorted]
    ds_ = 1.0 / (ws - 1.0) - 1.0 / ws
    b1s = host["bias_eff_cls"][t_sorted]
    e1s = np.exp(10.0 * diag)
    S = A - e1s + ds_ * ws * (S2 - e1s)
    P = 10.0 * (S3 - K * b1s - diag) - 10.0 * K
    val = P / K - np.log(S)
    return np.float32(-val.mean())


def _run(inputs, trace=False, **kw):
    in_maps, host = _prepare(**inputs)
    nc = _get_nc(host["eqw"], host["corr_last"], host["woff"])
    res = run_bass_kernel_spmd(nc, in_maps, list(range(NCORES)), trace=trace, **kw)
    loss = _combine(res.results, host)
    return loss, res


def kernel(**inputs):
    loss, _ = _run(inputs)
    return loss


# revision 19
# speedup vs baseline: 1.2566x; 1.0958x over previous
"""Trainium2 Bass kernel for the class-balanced supervised-contrastive loss.

Math (reference semantics, shift-invariant form with constant shift 10):
  l_ij = (f_i . g_j) / T,  T = 0.1, g = [features; centers; features_ood]
  E_ij = exp(l_ij - 10)
  S_i  = sum_{j != i} E_ij / (w_j - eq_ij)        (w_j = class count, eq = label match)
  P_i  = sum_{j != i} eq_ij (l_ij - 10)
  loss = -mean_i( P_i / K_i - log S_i ),  K_i = batch count of class t_i

Key identity: for matched columns j (eq_ij = 1) the class equals t_i, so w_j is a
per-row constant w*. The device therefore only needs, per row:
  A_i  = sum_j exp(10*(r_ij + bias1_j))      bias1_j = (ln(1/w_j) - 10)/10
  S2_i = sum_j eq_ij * E1_ij                 (E1 = the summand of A)
  S3_i = sum_j eq_ij * psum_ij               (psum = 1024*(r + bias1))
  diag = psum_ii                             (self column, for exclusion terms)
Everything else is O(B) host math.

Matmuls run in fp8 e4m3 DoubleRow mode (0.5 PE cycles/col, two K-slices per
instruction). Operands are scaled by 32 on the host, so psum = 1024*r and the
ACT exp uses scale 10/1024. Columns outside the matched window use a single
fh.gh term (random ~2e-2 error per exp term, averages out over ~9k summed
terms); the matched window gets fh.gh + fl.gh + fh.gl (logit error ~7e-4),
with the lo terms narrowed to the 256 columns where matches live. The
per-column bias enters as a K=2 bf16 matmul of ones against the hi/lo pair of
1024*bias1 (error ~1e-5). End-to-end numpy emulation: rel err 2.3e-5.

Device layout per core (rows sharded, 512 rows/core, 4 row tiles):
  per row tile: 5 psum groups of <=4 column chunks ([128,2048] = 4 banks,
  2-buffer ping-pong fills all 8 banks). Group 0 holds the window chunks.
  ACT: E1 = exp(psum*10/1024) with accum_out -> A partial per group
  DVE: scalar_tensor_tensor (ta == t_i) * {E1, psum} -> S2/S3; ident*psum diag
  Bulk g-matrix DMAs ride the otherwise-idle Pool (gpsimd) queue.
"""

import ml_dtypes
import numpy as np

import concourse.bass as bass
import concourse.mybir as mybir
import concourse.tile as tile
from concourse.bass_utils import run_bass_kernel_spmd

NCORES = 8
C, TEMP = 1000, 0.1
B, BO, D = 4096, 4096, 512
N = B + C + BO              # 9192
NPAD = 9216                 # 18 * 512
PAD = NPAD - N
NCH = NPAD // 512           # 18 column chunks
RPC = B // NCORES           # 512 rows per core
MT = RPC // 128             # 4 row tiles per core
SF = 32.0                   # fp8 operand scale; psum = SF*SF*(r + bias1)
PS = SF * SF                # 1024

F32 = mybir.dt.float32
BF16 = mybir.dt.bfloat16
FP8 = mybir.dt.float8e4
ALU = mybir.AluOpType
AF = mybir.ActivationFunctionType
BFNP = ml_dtypes.bfloat16
F8NP = ml_dtypes.float8_e4m3   # TRN FP8_EXP4: max +-240, then inf
DR = mybir.MatmulPerfMode.DoubleRow

# This walrus build accepts only one sync-wait command per engine instruction.
# Move surplus waits onto standalone EventSemaphore instructions just before
# the affected instruction (same engine, so blocking semantics are identical).
_SPLIT_SKIP = ("InstEventSemaphore",)


def _split_multi_waits(nc):
    n = 0
    for f in nc.m.functions:
        for bb in f.blocks:
            new = []
            for ins in bb.instructions:
                si = ins.sync_info
                if (
                    si is not None
                    and si.on_wait
                    and len(si.on_wait) > 1
                    and type(ins).__name__ not in _SPLIT_SKIP
                ):
                    waits = list(si.on_wait)
                    for w in waits[:-1]:
                        es = mybir.InstEventSemaphore(
                            name=f"wsplit_{n}",
                            engine=ins.engine,
                            sync_info=mybir.SyncInfo(on_wait=[w], on_update=[]),
                        )
                        n += 1
                        new.append(es)
                    ins.sync_info = mybir.SyncInfo(
                        on_wait=[waits[-1]], on_update=list(si.on_update)
                    )
                new.append(ins)
            bb.instructions = new
    return n


def _build_nc(eqw=2, corr_last=None, woff=None):
    assert eqw <= 4, "window chunks must fit in psum group 0"
    nc = bass.Bass()
    # host pre-tiles to the SBUF layout: col chunk ch lives at [128, 2048]
    # block ch with inner offset 512*k + j  (k = contraction slice)
    g8h = nc.declare_dram_parameter("g8h", [128, NCH * 2048], FP8, isOutput=False)
    g8l = nc.declare_dram_parameter("g8l", [128, eqw * 2048], FP8, isOutput=False)
    f8h = nc.declare_dram_parameter("f8h", [128, 2048], FP8, isOutput=False)
    f8l = nc.declare_dram_parameter("f8l", [128, 2048], FP8, isOutput=False)
    # [2, .]: row 0 = (ones128, hi of 1024*bias1), row 1 = (ones128, lo part)
    # bf16 pair only for the window chunks; cheap chunks take the bias as an
    # fp8 DoubleRow matmul of (8,1)-weighted ones against (hi/8, residual)
    cst = nc.declare_dram_parameter("cst", [2, 128 + 512 * eqw], BF16, isOutput=False)
    ones8 = nc.declare_dram_parameter("ones8", [1, 2, 128], FP8, isOutput=False)
    b8 = nc.declare_dram_parameter("b8", [1, 2, NPAD], FP8, isOutput=False)
    ta = nc.declare_dram_parameter("ta", [128, eqw * 512], F32, isOutput=False)
    tvec = nc.declare_dram_parameter("tvec", [128, MT], F32, isOutput=False)
    ident = nc.declare_dram_parameter("ident", [128, 128], F32, isOutput=False)
    out = nc.declare_dram_parameter("out", [128, 4 * MT], F32, isOutput=True)

    # column chunks packed into psum groups sized [4,4,4,3,3]: every ACT
    # drain is then >=1652ns, which covers the PE fill of the group two
    # slots ahead (2-buffer PSUM ping-pong never stalls the pipeline).
    # The two window chunks lead separate groups so each group's fill +
    # window-psum snapshot chain stays under one ACT drain. Emission order
    # per row tile puts the window groups (most input tensors) last and
    # the tail chunks (own DMA piece, smallest bias range) first.
    groups = [[0, 2, 3, 4], [1, 5, 6, 7], [8, 9, 10, 11],
              [12, 13, 14], [15, 16, 17]]
    emit_order = [4, 3, 2, 0, 1]
    NG = len(groups)

    with tile.TileContext(nc) as tc:
        with (
            tc.tile_pool(name="const", bufs=1) as const,
            tc.tile_pool(name="stats", bufs=1) as stats,
            tc.tile_pool(name="e1", bufs=3) as e1p,
            tc.tile_pool(name="scr", bufs=2) as scr,
            tc.tile_pool(name="psum", bufs=2, space="PSUM") as psp,
        ):
            # preload the exp spline table while DMAs stream in
            warm = const.tile([128, 1], F32)
            nc.vector.memset(warm[:], 0.0)
            nc.scalar.activation(warm[:], warm[:], AF.Exp)

            f8h_sb = const.tile([128, 4, 512], FP8)
            f8l_sb = const.tile([128, 4, 512], FP8)
            cst_sb = const.tile([2, 128 + 512 * eqw], BF16)
            ones8_sb = const.tile([1, 2, 128], FP8)
            b8_sb = const.tile([1, 2, NPAD], FP8)
            g8h_sb = const.tile([128, NCH * 4, 512], FP8)
            g8l_sb = const.tile([128, eqw * 4, 512], FP8)
            ta_sb = const.tile([128, eqw * 512], F32)
            tvec_sb = const.tile([128, MT], F32)
            ident_sb = const.tile([128, 128], F32)
            ones_sb = cst_sb[:, 0:128]
            brow_sb = cst_sb[:, 128 : 128 + 512 * eqw]

            # DMA split across the SP, Pool, and (idle-at-start) ACT queues,
            # each in the consumption order of the emitted psum groups.
            nc.sync.dma_start(out=f8h_sb[:], in_=f8h[:])
            nc.sync.dma_start(out=ones8_sb[:], in_=ones8[:])
            nc.sync.dma_start(out=b8_sb[:, :, 6144:], in_=b8[:, :, 6144:])
            nc.sync.dma_start(
                out=b8_sb[:, :, 4096:6144], in_=b8[:, :, 4096:6144]
            )
            nc.sync.dma_start(out=cst_sb[:], in_=cst[:])
            nc.sync.dma_start(
                out=b8_sb[:, :, 1024:4096], in_=b8[:, :, 1024:4096]
            )
            nc.sync.dma_start(out=f8l_sb[:], in_=f8l[:])
            nc.sync.dma_start(out=g8l_sb[:], in_=g8l[:])
            nc.sync.dma_start(out=tvec_sb[:], in_=tvec[:])
            nc.sync.dma_start(out=ident_sb[:], in_=ident[:])
            nc.sync.dma_start(out=ta_sb[:], in_=ta[:])

            for s, e in ((15, 18), (12, 15), (8, 12), (4, 8)):
                nc.gpsimd.dma_start(
                    out=g8h_sb[:, 4 * s : 4 * e, :],
                    in_=g8h[:, 2048 * s : 2048 * e],
                )
            nc.scalar.dma_start(out=g8h_sb[:, 0:16, :], in_=g8h[:, 0:8192])

            outsb = stats.tile([128, 4 * MT], F32)
            a_slot = [stats.tile([128, NG], F32, name=f"a{m}") for m in range(MT)]
            s2_slot = [stats.tile([128, eqw], F32, name=f"s2{m}") for m in range(MT)]
            s3_slot = [stats.tile([128, eqw], F32, name=f"s3{m}") for m in range(MT)]

            # the last row tile ends on a cheap group so the final-ACT ->
            # S2/reduce/DMA tail chain is as short as possible
            tail_order = [4, 3, 0, 1, 2]
            for m in range(MT):
                for gi in (tail_order if m == MT - 1 else emit_order):
                    chunks = groups[gi]
                    gw = 512 * len(chunks)
                    ps = psp.tile([128, 2048], F32)
                    for ci, ch in enumerate(chunks):
                        csl = ps[:, 512 * ci : 512 * (ci + 1)]
                        gcs = slice(512 * ch, 512 * (ch + 1))
                        is_win = ch < eqw
                        if is_win:
                            nc.tensor.matmul(
                                csl, ones_sb, brow_sb[:, gcs],
                                start=True, stop=False,
                            )
                        else:
                            nc.tensor.matmul(
                                csl, ones8_sb[:], b8_sb[:, :, gcs],
                                start=True, stop=False, perf_mode=DR,
                            )
                        for kp in range(2):
                            nc.tensor.matmul(
                                csl,
                                f8h_sb[:, 2 * kp : 2 * kp + 2, 128 * m : 128 * (m + 1)],
                                g8h_sb[:, 4 * ch + 2 * kp : 4 * ch + 2 * kp + 2, :],
                                start=False,
                                stop=(not is_win and kp == 1),
                                perf_mode=DR,
                            )
                        if is_win:
                            # lo correction terms, narrowed to the match window
                            if ch == 0 and woff is not None:
                                off, n = woff[m], 256
                            elif ch == eqw - 1 and corr_last is not None:
                                off, n = 0, corr_last
                            else:
                                off, n = 0, 512
                            for ti, (lh, rh) in enumerate(
                                ((f8l_sb, g8h_sb), (f8h_sb, g8l_sb))
                            ):
                                for kp in range(2):
                                    nc.tensor.matmul(
                                        ps[:, 512 * ci + off : 512 * ci + off + n],
                                        lh[:, 2 * kp : 2 * kp + 2,
                                           128 * m : 128 * (m + 1)],
                                        rh[:, 4 * ch + 2 * kp : 4 * ch + 2 * kp + 2,
                                           off : off + n],
                                        start=False,
                                        stop=(ti == 1 and kp == 1),
                                        perf_mode=DR,
                                    )
                    # PSUM readers from different engines serialize in
                    # emission order, so DVE takes one fp32 snapshot of the
                    # window columns (fits in the previous group's ACT drain)
                    # and every masked reduction reads SBUF afterwards; ACT is
                    # then the sole remaining psum reader and releases the
                    # buffer the moment it finishes.
                    nwin = sum(1 for ch in chunks if ch < eqw)
                    pc = None
                    if nwin:
                        pc = scr.tile([128, 512 * nwin], F32, tag="pc")
                        nc.vector.tensor_copy(
                            out=pc[:], in_=ps[:, 0 : 512 * nwin]
                        )
                    e1 = e1p.tile([128, 2048], BF16, name="e1", tag="e1")
                    nc.scalar.activation(
                        e1[:, 0:gw],
                        ps[:, 0:gw],
                        AF.Exp,
                        scale=10.0 / PS,
                        accum_out=a_slot[m][:, gi : gi + 1],
                    )
                    for ci, ch in enumerate(chunks):
                        if ch >= eqw:
                            continue
                        esl = slice(512 * ci, 512 * (ci + 1))
                        tsl = slice(512 * ch, 512 * (ch + 1))
                        sc3 = scr.tile([128, 512], BF16, tag="sc3")
                        nc.vector.scalar_tensor_tensor(
                            out=sc3[:],
                            in0=ta_sb[:, tsl],
                            scalar=tvec_sb[:, m : m + 1],
                            in1=pc[:, esl],
                            op0=ALU.is_equal,
                            op1=ALU.mult,
                            accum_out=s3_slot[m][:, ch : ch + 1],
                        )
                        if ch == 0:
                            # local row p's own column is chunk-0 column
                            # 128m+p: the psum diagonal of this [128,128]
                            # sub-block is the self dot-product (plus bias).
                            sd = scr.tile([128, 128], BF16, tag="sd")
                            nc.vector.scalar_tensor_tensor(
                                out=sd[:],
                                in0=ident_sb[:],
                                scalar=1.0,
                                in1=pc[:, 128 * m : 128 * (m + 1)],
                                op0=ALU.mult,
                                op1=ALU.mult,
                                accum_out=outsb[:, 4 * m + 3 : 4 * m + 4],
                            )
                        sc2 = scr.tile([128, 512], BF16, tag="sc2")
                        nc.vector.scalar_tensor_tensor(
                            out=sc2[:],
                            in0=ta_sb[:, tsl],
                            scalar=tvec_sb[:, m : m + 1],
                            in1=e1[:, esl],
                            op0=ALU.is_equal,
                            op1=ALU.mult,
                            accum_out=s2_slot[m][:, ch : ch + 1],
                        )

                nc.vector.tensor_reduce(
                    outsb[:, 4 * m : 4 * m + 1],
                    a_slot[m][:],
                    mybir.AxisListType.X,
                    ALU.add,
                )
                nc.vector.tensor_reduce(
                    outsb[:, 4 * m + 1 : 4 * m + 2],
                    s2_slot[m][:],
                    mybir.AxisListType.X,
                    ALU.add,
                )
                nc.vector.tensor_reduce(
                    outsb[:, 4 * m + 2 : 4 * m + 3],
                    s3_slot[m][:],
                    mybir.AxisListType.X,
                    ALU.add,
                )
            nc.sync.dma_start(out=out[:], in_=outsb[:])
    _split_multi_waits(nc)
    return nc


_nc_by_cfg = {}


def _get_nc(eqw, corr_last, woff):
    key = (eqw, corr_last, woff)
    if key not in _nc_by_cfg:
        _nc_by_cfg[key] = _build_nc(eqw, corr_last, woff)
    return _nc_by_cfg[key]


def _q8(x):
    return np.clip(np.asarray(x, np.float32), -240.0, 240.0).astype(F8NP)


def _prepare(centers1, features, targets, features_ood, pseudo_target_ood):
    """Host-side O(N log N) prep.

    Rows are globally sorted by class and sharded contiguously, so each
    core's 512 rows cover ~C/8 classes whose other members mostly live in
    the same core. Per core the g columns are permuted to
    [own 512 rows | all other same-class batch cols + own-class centers |
     rest bc cols | ood | pad], which confines every eq-match (and the
    diagonal, at column p for local row p) to the first EQW chunks. Only
    those chunks need the 3-term fp8 split and the masked S2/S3 reductions.
    """
    centers1 = np.asarray(centers1, np.float32)
    features = np.asarray(features, np.float32)
    features_ood = np.asarray(features_ood, np.float32)
    targets = np.asarray(targets).astype(np.int64)
    pseudo = np.asarray(pseudo_target_ood).astype(np.int64)

    tac = np.concatenate([targets, np.arange(C), pseudo])
    w_full = np.bincount(tac, minlength=C).astype(np.float64)

    # class-id label per g row (incl. centers/ood), and bias per g row
    lab = np.concatenate([targets, np.arange(C), np.full(BO, C, np.int64),
                          np.full(PAD, -1, np.int64)])
    bias1 = np.full(NPAD, -20.0, np.float64)
    bias1[:N] = -(np.log(w_full[tac]) + 10.0) / 10.0
    bs = PS * bias1
    b_h = bs.astype(BFNP)
    b_l = (bs - b_h.astype(np.float64)).astype(BFNP)

    g = np.concatenate(
        [features, centers1, features_ood, np.zeros((PAD, D), np.float32)], axis=0
    )
    g_h8 = _q8(SF * g)
    g_l8 = _q8(SF * g - g_h8.astype(np.float32))

    row_perm = np.argsort(targets, kind="stable")
    t_sorted = targets[row_perm]

    ident = np.eye(128, dtype=np.float32)
    ones2 = np.ones((2, 128), BFNP)

    # per-core column permutations
    perms = []
    eqw_need = 1
    mm_max = 0
    all_batch = np.arange(B)
    for c in range(NCORES):
        own = row_perm[RPC * c : RPC * (c + 1)]            # sorted by class
        tset = np.zeros(C + 1, bool)
        tset[t_sorted[RPC * c : RPC * (c + 1)]] = True
        in_own = np.zeros(B, bool)
        in_own[own] = True
        match_b = all_batch[tset[targets] & ~in_own]       # other cores' rows, own classes
        match_c = B + np.flatnonzero(tset[:C])             # centers of own classes
        matched = np.concatenate([match_b, match_c])
        rest_mask = np.ones(B + C, bool)
        rest_mask[own] = False
        rest_mask[matched] = False
        rest = np.flatnonzero(rest_mask)
        perm = np.concatenate(
            [own, matched, rest,
             np.arange(B + C, N),                          # ood
             np.arange(N, NPAD)]                           # pad
        )
        assert perm.shape == (NPAD,)
        perms.append(perm)
        eqw_need = max(eqw_need, -(-(RPC + len(matched)) // 512))
        mm_max = max(mm_max, RPC + len(matched))

    eqw = max(eqw_need, 2)  # chunks that must carry matches (expected 2)
    # columns the last window chunk must cover at full precision
    rem = mm_max - 512 * (eqw - 1)
    corr_last = 256 if rem <= 256 else None

    # chunk-0 correction windows per row-tile: row-tile m only matches own
    # columns whose classes occur in its rows — a narrow band around 128*m.
    WOFF = (0, 64, 192, 256)
    woff = WOFF
    for c in range(NCORES):
        tc_ = t_sorted[RPC * c : RPC * (c + 1)]
        for m in range(MT):
            cmin, cmax = tc_[128 * m], tc_[128 * m + 127]
            lo = np.searchsorted(tc_, cmin, side="left")
            hi = np.searchsorted(tc_, cmax, side="right")
            if not (WOFF[m] <= lo and hi <= WOFF[m] + 256):
                woff = None

    def tile_T(x):
        # [ncols, D] -> [128, (ncols/512)*2048] in the SBUF chunk layout:
        # block ch at ch*2048, inner offset 512*k + j  (k = dim-slice, j = col)
        nch = x.shape[0] // 512
        xt = np.ascontiguousarray(x.T)                     # [D, ncols]
        return np.ascontiguousarray(
            xt.reshape(4, 128, nch, 512).transpose(1, 2, 0, 3).reshape(128, nch * 2048)
        )

    ones8_c = np.empty((1, 2, 128), F8NP)
    ones8_c[0, 0] = F8NP(8.0)
    ones8_c[0, 1] = F8NP(1.0)

    in_maps = []
    for c in range(NCORES):
        perm = perms[c]
        bsp = bs[perm]
        b8_hi = _q8(bsp / 8.0)
        b8_lo = _q8(bsp - 8.0 * b8_hi.astype(np.float64))
        b8_c = np.ascontiguousarray(np.stack([b8_hi, b8_lo])[None])
        wsl = perm[: 512 * eqw]
        cst_c = np.ascontiguousarray(
            np.concatenate([ones2, np.stack([b_h[wsl], b_l[wsl]])], axis=1)
        )
        ta_p = lab[perm[: eqw * 512]].astype(np.float32)
        ta_bc = np.ascontiguousarray(np.broadcast_to(ta_p, (128, eqw * 512)))
        tvec_c = np.ascontiguousarray(
            t_sorted[RPC * c : RPC * (c + 1)].reshape(MT, 128).T.astype(np.float32)
        )
        in_maps.append(
            {
                "g8h": tile_T(g_h8[perm]),
                "g8l": tile_T(g_l8[perm[: eqw * 512]]),
                "f8h": tile_T(g_h8[perm[:RPC]]),
                "f8l": tile_T(g_l8[perm[:RPC]]),
                "cst": cst_c,
                "ones8": ones8_c,
                "b8": b8_c,
                "ta": ta_bc,
                "tvec": tvec_c,
                "ident": ident,
            }
        )

    # effective per-class bias as the device psum sees it (fp32 add of pair)
    cls_bias = PS * -(np.log(w_full) + 10.0) / 10.0
    cb_h = cls_bias.astype(BFNP)
    cb_l = (cls_bias - cb_h.astype(np.float64)).astype(BFNP)
    bias_eff_cls = (cb_h.astype(np.float64) + cb_l.astype(np.float64)) / PS

    host = {"t_sorted": t_sorted, "w_full": w_full, "bias_eff_cls": bias_eff_cls,
            "eqw": eqw, "corr_last": corr_last, "woff": woff}
    return in_maps, host


def _combine(results, host):
    t_sorted = host["t_sorted"]
    w_full = host["w_full"]
    cnt_batch = np.bincount(t_sorted, minlength=C).astype(np.float64)

    A = np.empty(B)
    S2 = np.empty(B)
    S3 = np.empty(B)
    diag = np.empty(B)
    for c in range(NCORES):
        o = np.asarray(results[c]["out"], np.float64)  # [128, 16]
        for m in range(MT):
            rs = slice(RPC * c + 128 * m, RPC * c + 128 * (m + 1))
            A[rs] = o[:, 4 * m]
            S2[rs] = o[:, 4 * m + 1]
            S3[rs] = o[:, 4 * m + 2] / PS
            diag[rs] = o[:, 4 * m + 3] / PS

    ws = w_full[t_sorted]
    K = cnt_batch[t_sorted]
    ds_ = 1.0 / (ws - 1.0) - 1.0 / ws
    b1s = host["bias_eff_cls"][t_sorted]
    e1s = np.exp(10.0 * diag)
    S = A - e1s + ds_ * ws * (S2 - e1s)
    P = 10.0 * (S3 - K * b1s - diag) - 10.0 * K
    val = P / K - np.log(S)
    return np.float32(-val.mean())


def _run(inputs, trace=False, **kw):
    in_maps, host = _prepare(**inputs)
    nc = _get_nc(host["eqw"], host["corr_last"], host["woff"])
    res = run_bass_kernel_spmd(nc, in_maps, list(range(NCORES)), trace=trace, **kw)
    loss = _combine(res.results, host)
    return loss, res


def kernel(**inputs):
    loss, _ = _run(inputs)
    return loss
